# revision 6
# baseline (speedup 1.0000x reference)
"""Distributed Trainium2 kernel for AssociativeSparseDistributedMemory.get_cliques.

Reference (B=128, INPUT=1024, VCAP=32768, K=32, ACAP=4096, K2=32):
  scores  = keys @ value_proj.T;  idx1 = top_k(scores, 32)
  p       = clique_encoder[idx1].sum(1)   (scale+normalize skipped: a positive
                                           per-row scale never changes a top-k set)
  scores2 = p @ assoc_proj.T;     idx2 = top_k(scores2, 32)
  out     = assoc_mem_value[idx2].sum(1)

Distribution over 8 cores (core m), v2:
  A : dummy 2KB AllGather issued at t~0 absorbs the CC-stream init barrier
      and first-collective cost concurrently with stage B'.
  B': value_proj rows [4096m, 4096(m+1)) -> score chunk [128, 4096], 512
      columns at a time; per 512-chunk keep only the top-8 (32 winners
      spread over 64 chunks: lambda=0.5/chunk, P(chunk holds >8) ~ 3e-9),
      via one max8 + find_index8 straight out of PSUM.
  C': single AllGather of the 64 aligned (value, index) pairs per core
      -> 512 global candidates everywhere; t32 = 32nd value; masked-index
      top-32 -> exact global top-32 indices giv on every core.
  E': indices -> int16 DGE wrapped layout, dma_gather pulls the 4096
      selected rows of the column-sharded clique_encoder (2KB rows) in 8
      calls of 512; tree-sum -> p chunk [128, 512].
  F : transpose p chunk, AllGather in partition-major layout [128, 512]
      so the gathered lhsT loads with 2KB descriptors.
  K : scores2 chunk = p @ assoc_proj[512m:512(m+1)].T (fp32); apT fully
      prefetched on the streaming queues during B'.
  L': local top-24 (lambda=4 winners/core, P(>24) ~ 1e-12), transpose s2
      chunk, single AllGather of [s2T chunk | cand24] -> every core holds
      all scores2 (transposed) + all candidates; t32_2 broadcast across
      partitions via a rank-1 PE matmul; w2T = (s2T_all >= t32_2) in bf16.
  Q : out chunk = w2 @ M[:, 4096m:4096(m+1)) in BF16 (selection exact in
      0/1 bf16; table quantization ~0.2%). Mb streams on the two
      never-blocking DMA queues (sync/scalar) behind vpTt and apT, so the
      stream prefetches through every collective wait.

  Queue discipline: sync+scalar queues carry ONLY unconditional streaming
  loads (vpTt, apT, Mb) so no semaphore-gated transfer can head-of-line
  block the prefetch; collective-dependent loads ride the vector queue,
  consts and the output ride the tensor queue.
"""

import numpy as np

B = 128
INPUT = 1024
VCAP = 32768
ACAP = 4096
K = 32
NCORES = 8
VSH = VCAP // NCORES      # 4096 value rows per core
ASH = ACAP // NCORES      # 512 assoc rows per core
K2L = 24                  # local stage-2 candidate count

_CACHE = {}

NEG = -1e30


def _build():
    import concourse.bass as bass
    import concourse.mybir as mybir
    import concourse.tile as tile
    from concourse import bacc
    from concourse.masks import make_identity

    f32 = mybir.dt.float32
    bf16 = mybir.dt.bfloat16
    i16 = mybir.dt.int16
    u16 = mybir.dt.uint16
    u8 = mybir.dt.uint8
    Alu = mybir.AluOpType

    nc = bacc.Bacc("TRN2", target_bir_lowering=False, debug=False,
                   num_devices=NCORES)

    # ---- kernel I/O ----
    keysT_d = nc.dram_tensor("keysT", [INPUT, B], f32, kind="ExternalInput")
    vpTt_d = nc.dram_tensor("vpTt", [8, 128, 8, 512], f32, kind="ExternalInput")
    Ecol_d = nc.dram_tensor("Ecol", [VCAP, ASH], f32, kind="ExternalInput")
    apT_d = nc.dram_tensor("apT", [128, 32, ASH], f32, kind="ExternalInput")
    Mb_d = nc.dram_tensor("Mb", [ACAP, VSH], bf16, kind="ExternalInput")
    rbase_d = nc.dram_tensor("rbase", [B, 1], f32, kind="ExternalInput")
    repl16_d = nc.dram_tensor("repl16", [128, 128], f32, kind="ExternalInput")
    dsel_d = nc.dram_tensor("dsel", [128, 8], f32, kind="ExternalInput")
    out_d = nc.dram_tensor("out", [B, VSH], f32, kind="ExternalOutput")

    # ---- internal DRAM ----
    dummy_in = nc.dram_tensor("dummy_in", [128, 4], f32)
    dummy_out = nc.dram_tensor("dummy_out", [128 * NCORES, 4], f32,
                               addr_space="Shared")
    cand1_in = nc.dram_tensor("cand1_in", [B, 128], f32)
    cand1_out = nc.dram_tensor("cand1_out", [B * NCORES, 128], f32,
                               addr_space="Shared")
    pag_in = nc.dram_tensor("pag_in", [128, 512], f32)
    pag_out = nc.dram_tensor("pag_out", [128 * NCORES, 512], f32,
                             addr_space="Shared")
    ag2_in = nc.dram_tensor("ag2_in", [128, 512 + K2L], f32)
    ag2_out = nc.dram_tensor("ag2_out", [128 * NCORES, 512 + K2L], f32,
                             addr_space="Shared")

    RG = [list(range(NCORES))]

    with tile.TileContext(nc) as tc:
        with (
            tc.tile_pool(name="const", bufs=1) as constp,
            tc.tile_pool(name="big", bufs=1) as bigp,
            tc.tile_pool(name="small", bufs=1) as smallp,
            tc.tile_pool(name="gat", bufs=2) as gatp,
            tc.tile_pool(name="rhsK", bufs=4) as rhsKp,
        ):
            psA_cm = tc.tile_pool(name="psA", bufs=2, space="PSUM")
            psA = psA_cm.__enter__()

            # ---- stage A: dummy collective to absorb CC-stream startup ----
            dmy = constp.tile([128, 4], f32)
            nc.vector.memset(dmy[:, :], 0.0)
            nc.scalar.dma_start(out=dummy_in[:, :], in_=dmy[:, :])
            nc.gpsimd.collective_compute(
                "AllGather", Alu.bypass, replica_groups=RG,
                ins=[dummy_in.ap().opt()], outs=[dummy_out.ap().opt()])

            ident = constp.tile([128, 128], f32)
            make_identity(nc, ident[:, :])
            keysT_sb = constp.tile([128, 8, 128], f32)
            for k in range(8):
                nc.sync.dma_start(out=keysT_sb[:, k, :],
                                  in_=keysT_d[k * 128:(k + 1) * 128, :])
            rbase = constp.tile([B, 1], f32)
            nc.scalar.dma_start(out=rbase[:, :], in_=rbase_d[:, :])
            repl16 = constp.tile([128, 128], f32)
            nc.scalar.dma_start(out=repl16[:, :], in_=repl16_d[:, :])
            dsel = constp.tile([128, 8], f32)
            nc.scalar.dma_start(out=dsel[:, :], in_=dsel_d[:, :])

            # ---- stage B': scores chunks + per-chunk top-8 from PSUM ----
            rhsBp_cm = tc.tile_pool(name="rhsB", bufs=2)
            rhsBp = rhsBp_cm.__enter__()
            cand = smallp.tile([B, 2, 64], f32)   # [:,0,:] vals, [:,1,:] idx
            for n in range(8):
                ps = psA.tile([128, 512], f32, tag="ps", name=f"psB{n}")
                rhs = rhsBp.tile([128, 8, 512], f32, tag="rhs", name=f"rB{n}")
                eng = nc.sync if n % 2 == 0 else nc.scalar
                eng.dma_start(out=rhs[:, :, :], in_=vpTt_d[n])
                for k in range(8):
                    nc.tensor.matmul(ps[:, :], keysT_sb[:, k, :], rhs[:, k, :],
                                     start=(k == 0), stop=(k == 7))
                idxn = smallp.tile([B, 8], u16, name=f"idxn{n}")
                nc.vector.max(out=cand[:, 0, n * 8:(n + 1) * 8], in_=ps[:, :])
                nc.vector.max_index(out=idxn[:, :],
                                    in_max=cand[:, 0, n * 8:(n + 1) * 8],
                                    in_values=ps[:, :])
                # global index = pos + rank_base + n*512
                nc.vector.tensor_scalar(
                    out=cand[:, 1, n * 8:(n + 1) * 8], in0=idxn[:, :],
                    scalar1=rbase[:, :], scalar2=float(n * 512),
                    op0=Alu.add, op1=Alu.add)

            # ---- stage K prefetch: apT rides the streaming queues just
            # behind vpTt (program order per queue) ----
            rhsK = []
            for j in range(4):
                rk = rhsKp.tile([128, 8, ASH], f32, tag="rhs", name=f"rK{j}")
                eng = nc.sync if j % 2 == 0 else nc.scalar
                eng.dma_start(out=rk[:, :, :],
                              in_=apT_d[:, j * 8:(j + 1) * 8, :])
                rhsK.append(rk)

            rhsBp_cm.__exit__(None, None, None)
            rhsQp_cm = tc.tile_pool(name="rhsQ", bufs=6)
            rhsQp = rhsQp_cm.__enter__()

            def topk32(vals, width, pool, pref, rounds=4):
                """mv [B, 8*rounds] = top values of vals [B, width] (desc)."""
                mv = pool.tile([B, 8 * rounds], f32, name=f"{pref}_mv",
                               tag=f"{pref}_mv")
                ms = pool.tile([B, width], f32, name=f"{pref}_ms",
                               tag=f"{pref}_ms")
                for r in range(rounds):
                    s = vals if r == 0 else ms[:, :]
                    nc.vector.max(out=mv[:, r * 8:(r + 1) * 8], in_=s)
                    if r < rounds - 1:
                        nc.vector.match_replace(
                            out=ms[:, :], in_to_replace=mv[:, r * 8:(r + 1) * 8],
                            in_values=s, imm_value=NEG)
                return mv

            # ---- stage C': ONE AllGather of (vals | idx), global merge ----
            nc.scalar.dma_start(
                out=cand1_in[:, :],
                in_=cand[:, :, :].rearrange("b t k -> b (t k)"))
            nc.gpsimd.collective_compute(
                "AllGather", Alu.bypass, replica_groups=RG,
                ins=[cand1_in.ap().opt()], outs=[cand1_out.ap().opt()])
            gvt = smallp.tile([B, NCORES, 64], f32)
            git = smallp.tile([B, NCORES, 64], f32)
            nc.scalar.dma_start(
                out=gvt[:, :, :],
                in_=cand1_out.ap()[:, 0:64].rearrange("(r b) k -> b r k",
                                                      r=NCORES, b=B))
            nc.scalar.dma_start(
                out=git[:, :, :],
                in_=cand1_out.ap()[:, 64:128].rearrange("(r b) k -> b r k",
                                                        r=NCORES, b=B))
            gvals = gvt[:, :, :].rearrange("b r k -> b (r k)")
            gidx = git[:, :, :].rearrange("b r k -> b (r k)")
            gmv = topk32(gvals, NCORES * 64, smallp, "gm")
            msk = smallp.tile([B, NCORES * 64], u8)
            nc.vector.tensor_scalar(out=msk[:, :], in0=gvals,
                                    scalar1=gmv[:, K - 1:K], scalar2=None,
                                    op0=Alu.is_ge)
            mi = smallp.tile([B, NCORES * 64], f32)
            nc.vector.memset(mi[:, :], -1.0)
            nc.vector.copy_predicated(out=mi[:, :], mask=msk[:, :], data=gidx)
            giv = topk32(mi[:, :], NCORES * 64, smallp, "gi")

            # ---- stage E': build the DGE wrapped index layout on-chip ----
            # idxs16[p', k*8+s0] = giv[16*s0 + p'%16, k].  Spread giv
            # diagonally into R[b, k, s0] (nonzero only when b//16 == s0),
            # then one matmul with the mod-16 replicator sums it into place.
            R = smallp.tile([128, K, 8], f32)
            nc.vector.tensor_tensor(
                out=R[:, :, :],
                in0=giv[:, :].broadcast_to([128, K, 8]),
                in1=dsel[:, None, :].broadcast_to([128, K, 8]),
                op=Alu.mult)
            psI = psA.tile([128, 256], f32, tag="ps", name="psI")
            nc.tensor.matmul(psI[:, :], repl16[:, :],
                             R[:, :, :].rearrange("p k s -> p (k s)"),
                             start=True, stop=True)
            idxs16 = smallp.tile([128, 256], i16)   # 4096 idxs / 16 lanes
            nc.vector.tensor_copy(idxs16[:, :], psI[:, :])
            # 8 gathers of 512 rows (33 SWDGE descriptors each); tree-sum
            # each batch of 4 slots while the next gather is in flight.
            p_chunk = smallp.tile([B, ASH], f32)
            for j in range(8):
                gath = gatp.tile([128, 4, ASH], f32, tag="gath", name=f"gath{j}")
                nc.gpsimd.dma_gather(
                    out_ap=gath[:, :, :], in_ap=Ecol_d.ap(),
                    idxs_ap=idxs16[:, j * 32:(j + 1) * 32],
                    num_idxs=512, num_idxs_reg=512, elem_size=ASH)
                a1 = gatp.tile([B, 2, ASH], f32, tag="a1", name=f"a1_{j}", bufs=1)
                nc.vector.tensor_tensor(out=a1[:, :, :], in0=gath[:, 0:2, :],
                                        in1=gath[:, 2:4, :], op=Alu.add)
                if j == 0:
                    nc.vector.tensor_tensor(out=p_chunk[:, :], in0=a1[:, 0, :],
                                            in1=a1[:, 1, :], op=Alu.add)
                else:
                    a2 = gatp.tile([B, ASH], f32, tag="a2", name=f"a2_{j}", bufs=1)
                    nc.vector.tensor_tensor(out=a2[:, :], in0=a1[:, 0, :],
                                            in1=a1[:, 1, :], op=Alu.add)
                    nc.vector.tensor_tensor(out=p_chunk[:, :], in0=p_chunk[:, :],
                                            in1=a2[:, :], op=Alu.add)

            # ---- stage F: transpose p chunk, AllGather partition-major ----
            pTc = smallp.tile([128, 4, 128], f32)
            for t in range(4):
                pt = psA.tile([128, 128], f32, tag="ps", name=f"ptJ{t}")
                nc.tensor.transpose(pt[:, :], p_chunk[:, t * 128:(t + 1) * 128],
                                    ident[:, :])
                nc.scalar.copy(pTc[:, t, :], pt[:, :])
            nc.scalar.dma_start(
                out=pag_in[:, :],
                in_=pTc[:, :, :].rearrange("p t c -> p (t c)"))
            nc.gpsimd.collective_compute(
                "AllGather", Alu.bypass, replica_groups=RG,
                ins=[pag_in.ap().opt()], outs=[pag_out.ap().opt()])
            big2 = bigp.tile([128, 32, 128], f32, tag="A")   # pT, later w2src
            nc.scalar.dma_start(
                out=big2[:, :, :].rearrange("p (m t) c -> p m t c",
                                            m=NCORES, t=4),
                in_=pag_out.ap().rearrange("(m p) (t c) -> p m t c",
                                           m=NCORES, p=128, t=4, c=128))

            # ---- stage K: scores2 chunk (fp32) ----
            s2 = smallp.tile([B, ASH], f32, tag="s2")
            psK = psA.tile([128, 512], f32, tag="ps", name="psK")
            for j in range(4):
                for k in range(8):
                    kk = j * 8 + k
                    nc.tensor.matmul(psK[:, :], big2[:, kk, :], rhsK[j][:, k, :],
                                     start=(kk == 0), stop=(kk == 31))
            nc.scalar.copy(s2[:, :], psK[:, :])

            # ---- stage L': local top-24 + s2 transpose, ONE AllGather ----
            pay2 = smallp.tile([128, 512 + K2L], f32, tag="pay2")
            scr2 = smallp.tile([B, ASH], f32, tag="scr2")
            for r in range(3):
                s = s2[:, :] if r == 0 else scr2[:, :]
                nc.vector.max(out=pay2[:, 512 + r * 8:512 + (r + 1) * 8], in_=s)
                if r < 2:
                    nc.vector.match_replace(
                        out=scr2[:, :],
                        in_to_replace=pay2[:, 512 + r * 8:512 + (r + 1) * 8],
                        in_values=s, imm_value=NEG)
            for t in range(4):
                pt = psA.tile([128, 128], f32, tag="ps", name=f"ptP{t}")
                nc.tensor.transpose(pt[:, :], s2[:, t * 128:(t + 1) * 128],
                                    ident[:, :])
                nc.scalar.copy(pay2[:, t * 128:(t + 1) * 128], pt[:, :])
            nc.scalar.dma_start(out=ag2_in[:, :], in_=pay2[:, :])
            nc.gpsimd.collective_compute(
                "AllGather", Alu.bypass, replica_groups=RG,
                ins=[ag2_in.ap().opt()], outs=[ag2_out.ap().opt()])

            # every core: all candidates -> t32_2; all s2T -> w2T mask
            cands2 = smallp.tile([B, NCORES, K2L], f32, tag="cs2")
            nc.scalar.dma_start(
                out=cands2[:, :, :],
                in_=ag2_out.ap()[:, 512:512 + K2L].rearrange(
                    "(m b) k -> b m k", m=NCORES, b=B))
            nc.scalar.dma_start(
                out=big2[:, :, :].rearrange("p (m t) c -> p m t c",
                                            m=NCORES, t=4),
                in_=ag2_out.ap()[:, 0:512].rearrange(
                    "(m p) (t c) -> p m t c", m=NCORES, p=128, t=4, c=128))
            mc2 = topk32(cands2[:, :, :].rearrange("b e k -> b (e k)"),
                         NCORES * K2L, smallp, "mc")
            # broadcast t32_2[b] across partitions: rank-128 stride-0 lhsT
            psb = psA.tile([128, 128], f32, tag="ps", name="psb")
            nc.tensor.matmul(psb[:, :],
                             mc2[:, K - 1:K].broadcast_to([128, 128]),
                             ident[:, :], start=True, stop=True)
            t32bc = smallp.tile([128, 128], f32, tag="t32bc")
            nc.scalar.copy(t32bc[:, :], psb[:, :])
            w2T = bigp.tile([128, 32, 128], bf16, tag="w2T")
            nc.vector.tensor_tensor(
                out=w2T[:, :, :], in0=big2[:, :, :],
                in1=t32bc[:, None, :].broadcast_to([128, 32, 128]),
                op=Alu.is_ge)

            # ---- stage Q: out chunk = w2 @ M_shard (bf16) ----
            psA_cm.__exit__(None, None, None)
            psQp_cm = tc.tile_pool(name="psQ", bufs=8, space="PSUM")
            psQp = psQp_cm.__enter__()
            out_sb = bigp.tile([B, VSH], f32, tag="B")
            psQ = [psQp.tile([128, 512], f32, tag="pq", name=f"psQ{n}")
                   for n in range(8)]
            for k in range(32):
                rhs = rhsQp.tile([128, VSH], bf16, tag="rhs", name=f"rQ{k}")
                eng = nc.sync if (k < 6 or k % 2 == 0) else nc.scalar
                eng.dma_start(out=rhs[:, :],
                              in_=Mb_d[k * 128:(k + 1) * 128, :])
                for n in range(8):
                    nc.tensor.matmul(psQ[n][:, :], w2T[:, k, :],
                                     rhs[:, n * 512:(n + 1) * 512],
                                     start=(k == 0), stop=(k == 31))
            for n in range(8):
                nc.scalar.copy(out_sb[:, n * 512:(n + 1) * 512], psQ[n][:, :])
                nc.sync.dma_start(out=out_d[:, n * 512:(n + 1) * 512],
                                  in_=out_sb[:, n * 512:(n + 1) * 512])
            psQp_cm.__exit__(None, None, None)
            rhsQp_cm.__exit__(None, None, None)

    nc.compile()
    return nc


def get_nc():
    if "nc" not in _CACHE:
        _CACHE["nc"] = _build()
    return _CACHE["nc"]


def make_in_maps(keys, value_proj, clique_encoder, assoc_proj, assoc_mem_value):
    import ml_dtypes
    keysT = np.ascontiguousarray(np.asarray(keys).T.astype(np.float32))
    value_proj = np.asarray(value_proj).astype(np.float32)
    clique_encoder = np.asarray(clique_encoder).astype(np.float32)
    assoc_proj = np.asarray(assoc_proj).astype(np.float32)
    Mb_full = np.asarray(assoc_mem_value).astype(ml_dtypes.bfloat16)
    bb, pp = np.meshgrid(np.arange(128), np.arange(128), indexing="ij")
    repl16 = (bb % 16 == pp % 16).astype(np.float32)
    dsel = (np.arange(128)[:, None] // 16 == np.arange(8)[None, :]).astype(np.float32)
    in_maps = []
    for m in range(NCORES):
        vpT = np.ascontiguousarray(
            value_proj[m * VSH:(m + 1) * VSH, :].T)        # [1024, 4096]
        # [n, p, k, c] so each n-chunk loads with one contiguous-per-partition DMA
        vpTt = np.ascontiguousarray(
            vpT.reshape(8, 128, 8, 512).transpose(2, 1, 0, 3))
        in_maps.append({
            "keysT": keysT,
            "vpTt": vpTt,
            "Ecol": np.ascontiguousarray(
                clique_encoder[:, m * ASH:(m + 1) * ASH]),
            "apT": np.ascontiguousarray(
                assoc_proj[m * ASH:(m + 1) * ASH, :].T
                .reshape(32, 128, ASH).transpose(1, 0, 2)),
            "Mb": np.ascontiguousarray(Mb_full[:, m * VSH:(m + 1) * VSH]),
            "rbase": np.full((B, 1), m * VSH, np.float32),
            "repl16": repl16,
            "dsel": dsel,
        })
    return in_maps


def kernel(keys, value_proj, clique_encoder, assoc_proj, assoc_mem_value,
           **run_kwargs):
    from concourse.bass_utils import run_bass_kernel_spmd

    nc = get_nc()
    in_maps = make_in_maps(keys, value_proj, clique_encoder, assoc_proj,
                           assoc_mem_value)
    res = run_bass_kernel_spmd(nc, in_maps, core_ids=list(range(NCORES)),
                               **run_kwargs)
    out = np.concatenate([np.asarray(res.results[m]["out"])
                          for m in range(NCORES)], axis=1)
    _CACHE["last_result"] = res
    return out


# revision 12
# speedup vs baseline: 1.1204x; 1.1204x over previous
"""Distributed Trainium2 kernel for AssociativeSparseDistributedMemory.get_cliques.

Reference (B=128, INPUT=1024, VCAP=32768, K=32, ACAP=4096, K2=32):
  scores  = keys @ value_proj.T;  idx1 = top_k(scores, 32)
  p       = clique_encoder[idx1].sum(1)   (scale+normalize skipped: a positive
                                           per-row scale never changes a top-k set)
  scores2 = p @ assoc_proj.T;     idx2 = top_k(scores2, 32)
  out     = assoc_mem_value[idx2].sum(1)

Distribution over 8 cores (core m), v2:
  A : dummy 2KB AllGather issued at t~0 absorbs the CC-stream init barrier
      and first-collective cost concurrently with stage B'.
  B': value_proj rows [4096m, 4096(m+1)) -> score chunk [128, 4096], 512
      columns at a time; per 512-chunk keep only the top-8 (32 winners
      spread over 64 chunks: lambda=0.5/chunk, P(chunk holds >8) ~ 3e-9),
      via one max8 + find_index8 straight out of PSUM.
  C': single AllGather of the 64 aligned (value, index) pairs per core
      -> 512 global candidates everywhere; t32 = 32nd value; masked-index
      top-32 -> exact global top-32 indices giv on every core.
  E': indices -> int16 DGE wrapped layout, dma_gather pulls the 4096
      selected rows of the column-sharded clique_encoder (2KB rows) in 8
      calls of 512; tree-sum -> p chunk [128, 512].
  F : transpose p chunk, AllGather in partition-major layout [128, 512]
      so the gathered lhsT loads with 2KB descriptors.
  K : scores2 chunk = p @ assoc_proj[512m:512(m+1)].T (fp32); apT fully
      prefetched on the streaming queues during B'.
  L': local top-24 (lambda=4 winners/core, P(>24) ~ 1e-12), transpose s2
      chunk, single AllGather of [s2T chunk | cand24] -> every core holds
      all scores2 (transposed) + all candidates; t32_2 broadcast across
      partitions via a rank-1 PE matmul; w2T = (s2T_all >= t32_2) in bf16.
  Q : out chunk = w2 @ M[:, 4096m:4096(m+1)) in BF16 (selection exact in
      0/1 bf16; table quantization ~0.2%). Mb streams on the two
      never-blocking DMA queues (sync/scalar) behind vpTt and apT, so the
      stream prefetches through every collective wait.

  Queue discipline: sync+scalar queues carry ONLY unconditional streaming
  loads (vpTt, apT, Mb) so no semaphore-gated transfer can head-of-line
  block the prefetch; collective-dependent loads ride the vector queue,
  consts and the output ride the tensor queue.
"""

import numpy as np

B = 128
INPUT = 1024
VCAP = 32768
ACAP = 4096
K = 32
NCORES = 8
VSH = VCAP // NCORES      # 4096 value rows per core
ASH = ACAP // NCORES      # 512 assoc rows per core
K2L = 24                  # local stage-2 candidate count

_CACHE = {}

NEG = -1e30


def _build():
    import concourse.bass as bass
    import concourse.mybir as mybir
    import concourse.tile as tile
    from concourse import bacc
    from concourse.masks import make_identity

    f32 = mybir.dt.float32
    bf16 = mybir.dt.bfloat16
    i16 = mybir.dt.int16
    u16 = mybir.dt.uint16
    u8 = mybir.dt.uint8
    Alu = mybir.AluOpType

    nc = bacc.Bacc("TRN2", target_bir_lowering=False, debug=False,
                   num_devices=NCORES)

    # ---- kernel I/O ----
    keysT_d = nc.dram_tensor("keysT", [INPUT, B], f32, kind="ExternalInput")
    vpTt_d = nc.dram_tensor("vpTt", [8, 128, 8, 512], f32, kind="ExternalInput")
    Ecol_d = nc.dram_tensor("Ecol", [VCAP, ASH], f32, kind="ExternalInput")
    apT_d = nc.dram_tensor("apT", [128, 32, ASH], f32, kind="ExternalInput")
    Mb_d = nc.dram_tensor("Mb", [ACAP, VSH], bf16, kind="ExternalInput")
    rbase_d = nc.dram_tensor("rbase", [B, 1], f32, kind="ExternalInput")
    repl16_d = nc.dram_tensor("repl16", [128, 128], f32, kind="ExternalInput")
    dsel_d = nc.dram_tensor("dsel", [128, 8], f32, kind="ExternalInput")
    out_d = nc.dram_tensor("out", [B, VSH], f32, kind="ExternalOutput")

    # ---- internal DRAM ----
    dummy_in = nc.dram_tensor("dummy_in", [128, 4], f32)
    dummy_out = nc.dram_tensor("dummy_out", [128 * NCORES, 4], f32,
                               addr_space="Shared")
    cand1_in = nc.dram_tensor("cand1_in", [B, 128], f32)
    cand1_out = nc.dram_tensor("cand1_out", [B * NCORES, 128], f32,
                               addr_space="Shared")
    pag_in = nc.dram_tensor("pag_in", [128, 512], f32)
    pag_out = nc.dram_tensor("pag_out", [128 * NCORES, 512], f32,
                             addr_space="Shared")
    ag2_in = nc.dram_tensor("ag2_in", [128, 512 + K2L], f32)
    ag2_out = nc.dram_tensor("ag2_out", [128 * NCORES, 512 + K2L], f32,
                             addr_space="Shared")

    RG = [list(range(NCORES))]

    with tile.TileContext(nc) as tc:
        with (
            tc.tile_pool(name="const", bufs=1) as constp,
            tc.tile_pool(name="big", bufs=1) as bigp,
            tc.tile_pool(name="small", bufs=1) as smallp,
            tc.tile_pool(name="gat", bufs=2) as gatp,
            tc.tile_pool(name="rhsK", bufs=2) as rhsKp,
        ):
            psA_cm = tc.tile_pool(name="psA", bufs=2, space="PSUM")
            psA = psA_cm.__enter__()

            # ---- stage A: dummy collective to absorb CC-stream startup ----
            dmy = constp.tile([128, 4], f32)
            nc.vector.memset(dmy[:, :], 0.0)
            nc.scalar.dma_start(out=dummy_in[:, :], in_=dmy[:, :])
            nc.gpsimd.collective_compute(
                "AllGather", Alu.bypass, replica_groups=RG,
                ins=[dummy_in.ap().opt()], outs=[dummy_out.ap().opt()])

            ident = constp.tile([128, 128], f32)
            make_identity(nc, ident[:, :])
            keysT_sb = constp.tile([128, 8, 128], f32)
            for k in range(8):
                nc.sync.dma_start(out=keysT_sb[:, k, :],
                                  in_=keysT_d[k * 128:(k + 1) * 128, :])
            rbase = constp.tile([B, 1], f32)
            nc.scalar.dma_start(out=rbase[:, :], in_=rbase_d[:, :])
            repl16 = constp.tile([128, 128], f32)
            nc.scalar.dma_start(out=repl16[:, :], in_=repl16_d[:, :])
            dsel = constp.tile([128, 8], f32)
            nc.scalar.dma_start(out=dsel[:, :], in_=dsel_d[:, :])

            # ---- stage B': scores chunks + per-chunk top-8 from PSUM ----
            rhsBp_cm = tc.tile_pool(name="rhsB", bufs=2)
            rhsBp = rhsBp_cm.__enter__()
            cand = smallp.tile([B, 2, 64], f32)   # [:,0,:] vals, [:,1,:] idx
            for n in range(8):
                ps = psA.tile([128, 512], f32, tag="ps", name=f"psB{n}")
                rhs = rhsBp.tile([128, 8, 512], f32, tag="rhs", name=f"rB{n}")
                eng = nc.sync if n % 2 == 0 else nc.scalar
                for k in range(8):
                    eng.dma_start(out=rhs[:, k, :], in_=vpTt_d[n][:, k, :])
                for k in range(8):
                    nc.tensor.matmul(ps[:, :], keysT_sb[:, k, :], rhs[:, k, :],
                                     start=(k == 0), stop=(k == 7))
                idxn = smallp.tile([B, 8], u16, name=f"idxn{n}")
                nc.vector.max(out=cand[:, 0, n * 8:(n + 1) * 8], in_=ps[:, :])
                nc.vector.max_index(out=idxn[:, :],
                                    in_max=cand[:, 0, n * 8:(n + 1) * 8],
                                    in_values=ps[:, :])
                # global index = pos + rank_base + n*512
                nc.vector.tensor_scalar(
                    out=cand[:, 1, n * 8:(n + 1) * 8], in0=idxn[:, :],
                    scalar1=rbase[:, :], scalar2=float(n * 512),
                    op0=Alu.add, op1=Alu.add)

            # ---- stage K prefetch: apT rides the streaming queues just
            # behind vpTt (program order per queue) ----
            rhsK = []
            for j in range(4):
                rk = rhsKp.tile([128, 8, ASH], f32, tag="rhs", name=f"rK{j}")
                eng = nc.sync if j % 2 == 0 else nc.scalar
                eng.dma_start(out=rk[:, :, :],
                              in_=apT_d[:, j * 8:(j + 1) * 8, :])
                rhsK.append(rk)

            rhsBp_cm.__exit__(None, None, None)
            rhsQp_cm = tc.tile_pool(name="rhsQ", bufs=8)
            rhsQp = rhsQp_cm.__enter__()

            def topk32(vals, width, pool, pref, rounds=4):
                """mv [B, 8*rounds] = top values of vals [B, width] (desc)."""
                mv = pool.tile([B, 8 * rounds], f32, name=f"{pref}_mv",
                               tag=f"{pref}_mv")
                ms = pool.tile([B, width], f32, name=f"{pref}_ms",
                               tag=f"ms{width}")
                for r in range(rounds):
                    s = vals if r == 0 else ms[:, :]
                    nc.vector.max(out=mv[:, r * 8:(r + 1) * 8], in_=s)
                    if r < rounds - 1:
                        nc.vector.match_replace(
                            out=ms[:, :], in_to_replace=mv[:, r * 8:(r + 1) * 8],
                            in_values=s, imm_value=NEG)
                return mv

            # ---- stage C': ONE AllGather of (vals | idx), global merge ----
            nc.scalar.dma_start(
                out=cand1_in[:, :],
                in_=cand[:, :, :].rearrange("b t k -> b (t k)"))
            nc.gpsimd.collective_compute(
                "AllGather", Alu.bypass, replica_groups=RG,
                ins=[cand1_in.ap().opt()], outs=[cand1_out.ap().opt()])
            gvgi = smallp.tile([B, NCORES, 128], f32)
            nc.scalar.dma_start(
                out=gvgi[:, :, :],
                in_=cand1_out.ap().rearrange("(r b) x -> b r x",
                                             r=NCORES, b=B))
            gvflat = smallp.tile([B, NCORES * 64], f32)
            giflat = smallp.tile([B, NCORES * 64], f32)
            nc.vector.tensor_copy(
                gvflat[:, :].rearrange("b (r k) -> b r k", r=NCORES, k=64),
                gvgi[:, :, 0:64])
            nc.vector.tensor_copy(
                giflat[:, :].rearrange("b (r k) -> b r k", r=NCORES, k=64),
                gvgi[:, :, 64:128])
            gvals = gvflat[:, :]
            gidx = giflat[:, :]
            gmv = topk32(gvals, NCORES * 64, smallp, "gm")
            msk = smallp.tile([B, NCORES * 64], u8)
            nc.vector.tensor_scalar(out=msk[:, :], in0=gvals,
                                    scalar1=gmv[:, K - 1:K], scalar2=None,
                                    op0=Alu.is_ge)
            mi = smallp.tile([B, NCORES * 64], f32)
            nc.vector.memset(mi[:, :], -1.0)
            nc.vector.copy_predicated(out=mi[:, :], mask=msk[:, :], data=gidx)
            giv = topk32(mi[:, :], NCORES * 64, smallp, "gi")

            # ---- stage E': build the DGE wrapped index layout on-chip ----
            # idxs16[p', k*8+s0] = giv[16*s0 + p'%16, k].  Spread giv
            # diagonally into R[b, k, s0] (nonzero only when b//16 == s0),
            # then one matmul with the mod-16 replicator sums it into place.
            R = smallp.tile([128, K, 8], f32)
            nc.vector.tensor_tensor(
                out=R[:, :, :],
                in0=giv[:, :].broadcast_to([128, K, 8]),
                in1=dsel[:, None, :].broadcast_to([128, K, 8]),
                op=Alu.mult)
            psI = psA.tile([128, 256], f32, tag="ps", name="psI")
            nc.tensor.matmul(psI[:, :], repl16[:, :],
                             R[:, :, :].rearrange("p k s -> p (k s)"),
                             start=True, stop=True)
            idxs16 = smallp.tile([128, 256], i16)   # 4096 idxs / 16 lanes
            nc.vector.tensor_copy(idxs16[:, :], psI[:, :])
            # 8 gathers of 512 rows (33 SWDGE descriptors each); tree-sum
            # each batch of 4 slots while the next gather is in flight.
            p_chunk = smallp.tile([B, ASH], f32)
            for j in range(4):
                gath = gatp.tile([128, 8, ASH], f32, tag="gath", name=f"gath{j}")
                nc.gpsimd.dma_gather(
                    out_ap=gath[:, :, :], in_ap=Ecol_d.ap(),
                    idxs_ap=idxs16[:, j * 64:(j + 1) * 64],
                    num_idxs=1024, num_idxs_reg=1024, elem_size=ASH)
                a1 = gatp.tile([B, 4, ASH], f32, tag="a1", name=f"a1_{j}", bufs=1)
                nc.vector.tensor_tensor(out=a1[:, :, :], in0=gath[:, 0:4, :],
                                        in1=gath[:, 4:8, :], op=Alu.add)
                a2 = gatp.tile([B, 2, ASH], f32, tag="a2", name=f"a2_{j}", bufs=1)
                nc.vector.tensor_tensor(out=a2[:, :, :], in0=a1[:, 0:2, :],
                                        in1=a1[:, 2:4, :], op=Alu.add)
                if j == 0:
                    nc.vector.tensor_tensor(out=p_chunk[:, :], in0=a2[:, 0, :],
                                            in1=a2[:, 1, :], op=Alu.add)
                else:
                    a3 = gatp.tile([B, ASH], f32, tag="a3", name=f"a3_{j}", bufs=1)
                    nc.vector.tensor_tensor(out=a3[:, :], in0=a2[:, 0, :],
                                            in1=a2[:, 1, :], op=Alu.add)
                    nc.vector.tensor_tensor(out=p_chunk[:, :], in0=p_chunk[:, :],
                                            in1=a3[:, :], op=Alu.add)

            # ---- stage F: transpose p chunk, AllGather partition-major ----
            pTc = smallp.tile([128, 4, 128], f32)
            for t in range(4):
                pt = psA.tile([128, 128], f32, tag="ps", name=f"ptJ{t}")
                nc.tensor.transpose(pt[:, :], p_chunk[:, t * 128:(t + 1) * 128],
                                    ident[:, :])
                nc.scalar.copy(pTc[:, t, :], pt[:, :])
            nc.scalar.dma_start(
                out=pag_in[:, :],
                in_=pTc[:, :, :].rearrange("p t c -> p (t c)"))
            nc.gpsimd.collective_compute(
                "AllGather", Alu.bypass, replica_groups=RG,
                ins=[pag_in.ap().opt()], outs=[pag_out.ap().opt()])
            big2 = bigp.tile([128, 32, 128], f32, tag="A")   # pT, later w2src
            nc.scalar.dma_start(
                out=big2[:, :, :].rearrange("p (m t) c -> p m t c",
                                            m=NCORES, t=4),
                in_=pag_out.ap().rearrange("(m p) (t c) -> p m t c",
                                           m=NCORES, p=128, t=4, c=128))

            # ---- stage K: scores2 chunk (fp32) ----
            s2 = smallp.tile([B, ASH], f32, tag="s2")
            psK = psA.tile([128, 512], f32, tag="ps", name="psK")
            for j in range(4):
                for k in range(8):
                    kk = j * 8 + k
                    nc.tensor.matmul(psK[:, :], big2[:, kk, :], rhsK[j][:, k, :],
                                     start=(kk == 0), stop=(kk == 31))
            nc.scalar.copy(s2[:, :], psK[:, :])

            # ---- stage L': local top-24 + s2 transpose, ONE AllGather ----
            pay2 = smallp.tile([128, 512 + K2L], f32, tag="pay2")
            scr2 = smallp.tile([B, ASH], f32, tag="scr2")
            for r in range(3):
                s = s2[:, :] if r == 0 else scr2[:, :]
                nc.vector.max(out=pay2[:, 512 + r * 8:512 + (r + 1) * 8], in_=s)
                if r < 2:
                    nc.vector.match_replace(
                        out=scr2[:, :],
                        in_to_replace=pay2[:, 512 + r * 8:512 + (r + 1) * 8],
                        in_values=s, imm_value=NEG)
            for t in range(4):
                pt = psA.tile([128, 128], f32, tag="ps", name=f"ptP{t}")
                nc.tensor.transpose(pt[:, :], s2[:, t * 128:(t + 1) * 128],
                                    ident[:, :])
                nc.scalar.copy(pay2[:, t * 128:(t + 1) * 128], pt[:, :])
            nc.scalar.dma_start(out=ag2_in[:, :], in_=pay2[:, :])
            nc.gpsimd.collective_compute(
                "AllGather", Alu.bypass, replica_groups=RG,
                ins=[ag2_in.ap().opt()], outs=[ag2_out.ap().opt()])

            # every core: all candidates -> t32_2; all s2T -> w2T mask
            cands2 = smallp.tile([B, NCORES, K2L], f32, tag="cs2")
            nc.scalar.dma_start(
                out=cands2[:, :, :],
                in_=ag2_out.ap()[:, 512:512 + K2L].rearrange(
                    "(m b) k -> b m k", m=NCORES, b=B))
            for h, heng in ((0, nc.sync), (1, nc.scalar)):
                heng.dma_start(
                    out=big2[:, :, :].rearrange("p (m t) c -> p m t c",
                                                m=NCORES, t=4)[:, h * 4:(h + 1) * 4],
                    in_=ag2_out.ap()[:, 0:512].rearrange(
                        "(m p) (t c) -> p m t c", m=NCORES, p=128, t=4,
                        c=128)[:, h * 4:(h + 1) * 4])
            mc2 = topk32(cands2[:, :, :].rearrange("b e k -> b (e k)"),
                         NCORES * K2L, smallp, "mc")
            # broadcast t32_2[b] across partitions: rank-128 stride-0 lhsT
            psb = psA.tile([128, 128], f32, tag="ps", name="psb")
            nc.tensor.matmul(psb[:, :],
                             mc2[:, K - 1:K].broadcast_to([128, 128]),
                             ident[:, :], start=True, stop=True)
            t32bc = smallp.tile([128, 128], f32, tag="t32bc")
            nc.scalar.copy(t32bc[:, :], psb[:, :])
            w2T = bigp.tile([128, 32, 128], bf16, tag="w2T")
            nc.vector.tensor_tensor(
                out=w2T[:, :, :], in0=big2[:, :, :],
                in1=t32bc[:, None, :].broadcast_to([128, 32, 128]),
                op=Alu.is_ge)

            # ---- stage Q: out chunk = w2 @ M_shard (bf16) ----
            psA_cm.__exit__(None, None, None)
            psQp_cm = tc.tile_pool(name="psQ", bufs=8, space="PSUM")
            psQp = psQp_cm.__enter__()
            psQ = [psQp.tile([128, 512], f32, tag="pq", name=f"psQ{n}")
                   for n in range(8)]
            for kp in range(16):
                k0, k1 = 2 * kp, 2 * kp + 1
                rr = []
                for k in (k0, k1):
                    rhs = rhsQp.tile([128, VSH], bf16, tag="rhs", name=f"rQ{k}")
                    eng = nc.sync if (k < 6 or k % 2 == 0) else nc.scalar
                    eng.dma_start(out=rhs[:, :],
                                  in_=Mb_d[k * 128:(k + 1) * 128, :])
                    rr.append(rhs)
                # alternate the two weight sets so every LDWEIGHTS targets
                # the background buffer while the other weight's MM streams
                for n in range(8):
                    nc.tensor.matmul(psQ[n][:, :], w2T[:, k0, :],
                                     rr[0][:, n * 512:(n + 1) * 512],
                                     start=(k0 == 0), stop=False)
                    nc.tensor.matmul(psQ[n][:, :], w2T[:, k1, :],
                                     rr[1][:, n * 512:(n + 1) * 512],
                                     start=False, stop=(k1 == 31))
            outp_cm = tc.tile_pool(name="outp", bufs=2)
            outp = outp_cm.__enter__()
            for n in range(8):
                ob = outp.tile([128, 512], f32, tag="ob", name=f"ob{n}")
                nc.scalar.copy(ob[:, :], psQ[n][:, :])
                eng = nc.sync if n % 2 == 0 else nc.scalar
                eng.dma_start(out=out_d[:, n * 512:(n + 1) * 512],
                              in_=ob[:, :])
            outp_cm.__exit__(None, None, None)
            psQp_cm.__exit__(None, None, None)
            rhsQp_cm.__exit__(None, None, None)

    nc.compile()
    return nc


def get_nc():
    if "nc" not in _CACHE:
        _CACHE["nc"] = _build()
    return _CACHE["nc"]


def make_in_maps(keys, value_proj, clique_encoder, assoc_proj, assoc_mem_value):
    import ml_dtypes
    keysT = np.ascontiguousarray(np.asarray(keys).T.astype(np.float32))
    value_proj = np.asarray(value_proj).astype(np.float32)
    clique_encoder = np.asarray(clique_encoder).astype(np.float32)
    assoc_proj = np.asarray(assoc_proj).astype(np.float32)
    Mb_full = np.asarray(assoc_mem_value).astype(ml_dtypes.bfloat16)
    bb, pp = np.meshgrid(np.arange(128), np.arange(128), indexing="ij")
    repl16 = (bb % 16 == pp % 16).astype(np.float32)
    dsel = (np.arange(128)[:, None] // 16 == np.arange(8)[None, :]).astype(np.float32)
    in_maps = []
    for m in range(NCORES):
        vpT = np.ascontiguousarray(
            value_proj[m * VSH:(m + 1) * VSH, :].T)        # [1024, 4096]
        # [n, p, k, c] so each n-chunk loads with one contiguous-per-partition DMA
        vpTt = np.ascontiguousarray(
            vpT.reshape(8, 128, 8, 512).transpose(2, 1, 0, 3))
        in_maps.append({
            "keysT": keysT,
            "vpTt": vpTt,
            "Ecol": np.ascontiguousarray(
                clique_encoder[:, m * ASH:(m + 1) * ASH]),
            "apT": np.ascontiguousarray(
                assoc_proj[m * ASH:(m + 1) * ASH, :].T
                .reshape(32, 128, ASH).transpose(1, 0, 2)),
            "Mb": np.ascontiguousarray(Mb_full[:, m * VSH:(m + 1) * VSH]),
            "rbase": np.full((B, 1), m * VSH, np.float32),
            "repl16": repl16,
            "dsel": dsel,
        })
    return in_maps


def kernel(keys, value_proj, clique_encoder, assoc_proj, assoc_mem_value,
           **run_kwargs):
    from concourse.bass_utils import run_bass_kernel_spmd

    nc = get_nc()
    in_maps = make_in_maps(keys, value_proj, clique_encoder, assoc_proj,
                           assoc_mem_value)
    res = run_bass_kernel_spmd(nc, in_maps, core_ids=list(range(NCORES)),
                               **run_kwargs)
    out = np.concatenate([np.asarray(res.results[m]["out"])
                          for m in range(NCORES)], axis=1)
    _CACHE["last_result"] = res
    return out
